# revision 38
# baseline (speedup 1.0000x reference)
"""Trainium2 Bass kernel for nn_ConvNet (char-CNN word encoder + sentence conv + MLP).

Model (reference):
    vw   = word_emb[words]                                  # [W, D]
    ch   = chr_emb[words_in_char].transpose -> conv1d(k=3, pad=1) -> max over L
    u    = concat([vw, wch], axis=1)                        # [W, 2D]
    r    = max over W of conv1d(u.T, k=3, pad=1)            # [2D]
    out  = tanh(r @ w1.T + b1) @ w2.T + b2                  # [1, 2]

Char path: conv o embed is linear in the one-hot encoding of the char ids.
The per-tap response tables ET_k[c, :] = chr_emb[c, :] @ W_k.T are folded on
the host (weight preprocessing, scaled x64 into fp8 range) and the char conv
collapses to one-hot matmuls (one-hot fp8 is exact).

Dense polyphase layout: conv outputs are split by position parity and laid
out DENSELY (16 output cols per word per parity, no pad positions).  For
output col j of word w:
    y_even[j] = ET0[Le[j]] + ET1[cE[j]] + ET2[cO[j]]
    y_odd[j]  = ET0[cE[j]] + ET1[cO[j]] + ET2[Re[j]]
where cE/cO are the even/odd chars and Le = cO shifted right (pad at word
start), Re = cE shifted left (pad at word end).  cE/cO live in one two-slot
one-hot tile (feeding fp8 DoubleRow matmuls, 2 taps per pass); Le/Re in a
second tile (feeding the normal fp8 matmul).  All streams are host-built
one-hots, so word-boundary padding is baked in and every PSUM column is a
valid conv output: the per-word max is ONE fully-dense 4D tensor_reduce per
group straight from PSUM.

Sharding: data-parallel over words, 512+2 halo words per core.  The only
collective is an AllGather of the [128,4] sentence-conv partial maxes (the
8-way max is one local DVE reduce).  The tiny MLP is replicated.
"""

import sys

try:
    import concourse  # noqa: F401
except ImportError:
    sys.path.insert(0, "/opt/trn_rl_repo")

import numpy as np
import ml_dtypes

import concourse.bass as bass
import concourse.bacc as bacc
import concourse.tile as tile
from concourse import mybir
from concourse.bass_utils import run_bass_kernel_spmd

BF16 = ml_dtypes.bfloat16
FP8 = ml_dtypes.float8_e4m3

CORES = 8
D = 256
L = 32
HW = L // 2       # output cols per word per parity
GW = 32           # words per char-conv group (32*16 = 512 = PSUM bank)
ETS = 64.0        # fp8 scale for the ET response tables
SWI = True       # DoubleRowSwInterleave weight layout


def _shapes(W):
    WPC = W // CORES          # real words per core
    NW = WPC + 2              # + 1 halo word each side
    S = NW * HW               # parity-stream length (16B-aligned: HW=16)
    G = -(-NW // 128)         # word-gather groups of 128
    return WPC, NW, S, G


def build(W):
    WPC, NW, S, G = _shapes(W)
    f32 = mybir.dt.float32
    bf16 = mybir.dt.bfloat16
    f8 = mybir.dt.float8e4
    i32 = mybir.dt.int32

    nc = bacc.Bacc(num_devices=CORES)

    onehot = nc.declare_dram_parameter("onehot", [128, 2, S], f8, isOutput=False)
    wdr_shape = [128, 2, 2, 256] if SWI else [128, 2, 2, 2, 128]
    wdr = nc.declare_dram_parameter("wdr", wdr_shape, f8, isOutput=False)
    wn = nc.declare_dram_parameter("wn", [128, 2, 2, 128], f8, isOutput=False)
    widx = nc.declare_dram_parameter("widx", [128, G], i32, isOutput=False)
    wemb = nc.declare_dram_parameter("wemb", [50000, D], f32, isOutput=False)
    cbias = nc.declare_dram_parameter("cbias", [128, 2], f32, isOutput=False)
    wsT = nc.declare_dram_parameter("wsT", [128, 3, 4, 2 * D], bf16, isOutput=False)
    bsent = nc.declare_dram_parameter("bsent", [128, 4], f32, isOutput=False)
    w1t = nc.declare_dram_parameter("w1t", [128, 4, 8, 128], bf16, isOutput=False)
    b1t = nc.declare_dram_parameter("b1t", [128, 8], f32, isOutput=False)
    w2t = nc.declare_dram_parameter("w2t", [128, 8, 2], bf16, isOutput=False)
    b2t = nc.declare_dram_parameter("b2t", [2, 1], f32, isOutput=False)
    hsc = nc.declare_dram_parameter("hsc", [128, 2], bf16, isOutput=False)
    ident = nc.declare_dram_parameter("ident", [128, 128], f32, isOutput=False)
    out = nc.declare_dram_parameter("out", [2, 1], f32, isOutput=True)

    # char-conv groups
    groups = []
    g0 = 0
    while g0 < NW:
        groups.append((g0, min(GW, NW - g0)))
        g0 += GW

    sec_groups = [0, 1, 2, 3, 4, 6, 8, 10, 12, 14, 16]
    sec_groups = [sg for sg in sec_groups if sg < len(groups)]
    bounds = [0] + [groups[sg][0] * HW for sg in sec_groups[1:]] + [S]

    with tile.TileContext(nc) as tc:
        with (
            tc.tile_pool(name="const", bufs=1) as cpool,
            tc.tile_pool(name="gath", bufs=G) as gpool,
            tc.tile_pool(name="dram", bufs=1, space="DRAM") as dpool,
        ):
            # ---- critical-path loads: one-hots + char weights on 2 queues ------
            # one-hot center streams DMA'd on two queues; the side-tap streams
            # (oh2) are built ON DEVICE: Le = cO shifted right, Re = cE shifted
            # left, with word-boundary cols zeroed (a zero one-hot col
            # contributes exactly 0, the pad-char response).  Section bounds
            # are word-aligned, so each section's shift is self-contained.
            oh_sb = cpool.tile([128, 2, S], f8, name="oh_sb")
            oh2_sb = cpool.tile([128, 2, S], f8, name="oh2_sb")
            wdr_sb = cpool.tile(wdr_shape, f8, name="wdr_sb")
            wn_sb = cpool.tile([128, 2, 2, 128], f8, name="wn_sb")
            widx_sb = cpool.tile([128, G], i32)
            cbias_sb = cpool.tile([128, 2], f32)
            dum_in = dpool.tile([128, 2], f32)
            dum_out = nc.dram_tensor(
                "dum_out", [CORES, 128, 2], f32, addr_space="Shared"
            )
            def emit_oh2(a, b):
                # side-tap streams for section [a, b): shifted copies of the
                # center streams (SBUF->SBUF DMA) + word-boundary col zeroing
                nc.gpsimd.dma_start(
                    out=oh2_sb[:, 0, a + 1 : b], in_=oh_sb[:, 1, a : b - 1]
                )
                nc.sync.dma_start(
                    out=oh2_sb[:, 1, a : b - 1], in_=oh_sb[:, 0, a + 1 : b]
                )
                ov = oh2_sb[:, :, a:b].rearrange("p q (w t) -> p q w t", t=HW)
                nc.gpsimd.memset(ov[:, 0, :, 0:1], 0.0)
                nc.gpsimd.memset(ov[:, 1, :, HW - 1 : HW], 0.0)

            secs = list(zip(bounds[:-1], bounds[1:]))
            for si, (a, b) in enumerate(secs):
                eng = nc.scalar if si % 2 == 0 else nc.sync
                eng.dma_start(out=oh_sb[:, :, a:b], in_=onehot[:, :, a:b])
                if si > 0:
                    emit_oh2(*secs[si - 1])
                if si == 0:
                    nc.scalar.dma_start(out=wdr_sb[:], in_=wdr[:])
                    nc.scalar.dma_start(out=wn_sb[:], in_=wn[:])
                if si == 2:
                    nc.gpsimd.dma_start(out=widx_sb[:], in_=widx[:])
                    nc.gpsimd.dma_start(out=cbias_sb[:], in_=cbias[:])
                if si == 3:
                    # warm-up collective: wakes/arms ncfw early so the real
                    # AllGather at the end doesn't pay the cold-start latency.
                    # Runs on the CC engine async; nothing reads its output.
                    nc.gpsimd.dma_start(out=dum_in[:], in_=cbias_sb[:])
                    nc.gpsimd.collective_compute(
                        "AllGather",
                        mybir.AluOpType.bypass,
                        replica_groups=[list(range(CORES))],
                        ins=[dum_in[:]],
                        outs=[dum_out[:]],
                    )
            emit_oh2(*secs[-1])

            # ---- word-embedding gather (independent of char path) ---------------
            wrd_sb = []
            for g in range(G):
                wt = gpool.tile([128, D], f32, tag="wrd")
                nc.gpsimd.indirect_dma_start(
                    out=wt[:],
                    out_offset=None,
                    in_=wemb[:],
                    in_offset=bass.IndirectOffsetOnAxis(ap=widx_sb[:, g : g + 1], axis=0),
                )
                wrd_sb.append(wt)

            # ---- late constants: tiles declared here, DMAs emitted mid-loop ----
            # (so their HBM traffic does not contend with the one-hot streams)
            ident_sb = cpool.tile([128, 128], f32)
            hsc_sb = cpool.tile([128, 2], bf16)
            bsent_sb = cpool.tile([128, 4], f32)
            b1t_sb = cpool.tile([128, 8], f32)
            b2t_sb = cpool.tile([2, 1], f32)
            wsT_sb = cpool.tile([128, 3, 4, 2 * D], bf16)
            w1t_sb = cpool.tile([128, 4, 8, 128], bf16)
            w2t_sb = cpool.tile([128, 8, 2], bf16)

            def load_late_consts():
                nc.gpsimd.dma_start(out=ident_sb[:], in_=ident[:])
                nc.gpsimd.dma_start(out=hsc_sb[:], in_=hsc[:])
                nc.gpsimd.dma_start(out=bsent_sb[:], in_=bsent[:])
                nc.gpsimd.dma_start(out=b1t_sb[:], in_=b1t[:])
                nc.gpsimd.dma_start(out=b2t_sb[:], in_=b2t[:])
                nc.gpsimd.dma_start(out=wsT_sb[:], in_=wsT[:])
                nc.gpsimd.dma_start(out=w1t_sb[:], in_=w1t[:])
                nc.gpsimd.dma_start(out=w2t_sb[:], in_=w2t[:])

            # ---- char path: dense polyphase one-hot matmul + per-word max ------
            # rEO[:, 0:2, :] = even-parity maxes, [:, 2:4, :] = odd (x64 scale).
            rEO = cpool.tile([128, 4, NW], bf16, name="rEO")
            with nc.named_scope("char"):
              with (
                tc.tile_pool(name="pch", bufs=2, space="PSUM") as pch,
                tc.tile_pool(name="evac", bufs=4) as epool,
              ):
                for gi, (w0, nw) in enumerate(groups):
                    if gi == 11:
                        load_late_consts()
                    n = nw * HW
                    s0 = w0 * HW
                    py = pch.tile([128, 4, 512], f32, tag="py")
                    for par in range(2):
                        for m in range(2):
                            b = par * 2 + m
                            lhsT_dr = wdr_sb[:, par, m, :] if SWI else wdr_sb[:, par, :, m, :]
                            nc.tensor.matmul(
                                out=py[:, b, :n],
                                lhsT=lhsT_dr,
                                rhs=oh_sb[:, :, s0 : s0 + n],
                                start=True,
                                stop=False,
                                perf_mode=(
                                    mybir.MatmulPerfMode.DoubleRowSwInterleave
                                    if SWI
                                    else mybir.MatmulPerfMode.DoubleRow
                                ),
                            )
                            nc.tensor.matmul(
                                out=py[:, b, :n],
                                lhsT=wn_sb[:, par, m, :],
                                rhs=oh2_sb[:, par, s0 : s0 + n],
                                start=False,
                                stop=True,
                            )
                    # split per-word max: DVE reduces the even banks straight
                    # from PSUM while ScalarE evacuates the odd banks to bf16
                    # SBUF for a second (cheap, dense 16-bit) DVE reduce.
                    pv = py[:, 0:2, :n].rearrange("p b (w t) -> p b w t", t=HW)
                    nc.vector.tensor_reduce(
                        out=rEO[:, 0:2, w0 : w0 + nw],
                        in_=pv[:],
                        axis=mybir.AxisListType.X,
                        op=mybir.AluOpType.max,
                    )
                    yb = epool.tile([128, 2, GW * HW], bf16, tag="yb")
                    nc.scalar.activation(
                        out=yb[:, :, :n],
                        in_=py[:, 2:4, :n],
                        func=mybir.ActivationFunctionType.Copy,
                    )
                    yv = yb[:, :, :n].rearrange("p b (w t) -> p b w t", t=HW)
                    nc.vector.tensor_reduce(
                        out=rEO[:, 2:4, w0 : w0 + nw],
                        in_=yv[:],
                        axis=mybir.AxisListType.X,
                        op=mybir.AluOpType.max,
                    )

            # ---- assemble u^T [4][128, NW] bf16 --------------------------------
            # u[0..1] = word-embedding halves, u[2..3] = char halves (x64 scale,
            # undone by the 1/64 folded into the char-half sentence weights).
            u = [cpool.tile([128, G * 128], bf16, tag=f"u{j}", name=f"u{j}") for j in range(4)]
            for m in range(2):
                nc.vector.tensor_tensor(
                    out=u[2 + m][:, :NW],
                    in0=rEO[:, m, :],
                    in1=rEO[:, 2 + m, :],
                    op=mybir.AluOpType.max,
                )
                nc.vector.tensor_scalar(
                    out=u[2 + m][:, :NW],
                    in0=u[2 + m][:, :NW],
                    scalar1=cbias_sb[:, m : m + 1],
                    scalar2=None,
                    op0=mybir.AluOpType.add,
                )
            # halo columns: scale by 0/1 (core 0 left, core 7 right)
            for j in (2, 3):
                nc.vector.tensor_tensor(
                    out=u[j][:, 0:1], in0=u[j][:, 0:1], in1=hsc_sb[:, 0:1],
                    op=mybir.AluOpType.mult,
                )
                nc.vector.tensor_tensor(
                    out=u[j][:, NW - 1 : NW], in0=u[j][:, NW - 1 : NW],
                    in1=hsc_sb[:, 1:2], op=mybir.AluOpType.mult,
                )
            # word half: transpose gathered rows [word, ch] -> [ch, word]
            with tc.tile_pool(name="ptp", bufs=2, space="PSUM") as ptp:
                for g in range(G):
                    w = min(128, NW - g * 128)
                    for cc in range(2):
                        tp = ptp.tile([128, 128], f32, tag="tp")
                        nc.tensor.transpose(
                            out=tp[:],
                            in_=wrd_sb[g][:, cc * 128 : (cc + 1) * 128],
                            identity=ident_sb[:],
                        )
                        nc.scalar.activation(
                            out=u[cc][:, g * 128 : g * 128 + w],
                            in_=tp[:, :w],
                            func=mybir.ActivationFunctionType.Copy,
                        )

            # ---- sentence conv over the word axis + local max ------------------
            rloc = cpool.tile([128, 4], f32)
            with tc.tile_pool(name="psn", bufs=4, space="PSUM") as psn:
                for m in range(4):
                    ps = psn.tile([128, WPC], f32, tag="ps")
                    first = True
                    for k in range(3):
                        for kc in range(4):
                            nc.tensor.matmul(
                                out=ps[:],
                                lhsT=wsT_sb[:, k, kc, m * 128 : (m + 1) * 128],
                                rhs=u[kc][:, k : k + WPC],
                                start=first,
                                stop=(k == 2 and kc == 3),
                            )
                            first = False
                    nc.vector.tensor_reduce(
                        out=rloc[:, m : m + 1],
                        in_=ps[:],
                        axis=mybir.AxisListType.X,
                        op=mybir.AluOpType.max,
                    )

            # ---- AllGather of the partial channel maxes + local 8-way max ------
            cc_in = dpool.tile([128, 4], f32)
            cc_out = nc.dram_tensor("cc_out", [CORES, 128, 4], f32, addr_space="Shared")
            nc.gpsimd.dma_start(out=cc_in[:], in_=rloc[:])
            nc.gpsimd.collective_compute(
                "AllGather",
                mybir.AluOpType.bypass,
                replica_groups=[list(range(CORES))],
                ins=[cc_in[:]],
                outs=[cc_out[:]],
            )
            rg = cpool.tile([128, CORES, 4], f32)
            nc.sync.dma_start(out=rg[:], in_=cc_out[:].rearrange("r p f -> p r f"))
            rmax = cpool.tile([128, 4], f32)
            nc.vector.tensor_reduce(
                out=rmax[:],
                in_=rg[:].rearrange("p r f -> p f r"),
                axis=mybir.AxisListType.X,
                op=mybir.AluOpType.max,
            )
            r_sb = cpool.tile([128, 4], bf16)
            nc.vector.tensor_tensor(
                out=r_sb[:], in0=rmax[:], in1=bsent_sb[:], op=mybir.AluOpType.add
            )

            # ---- MLP (replicated on every core) --------------------------------
            h_sb = cpool.tile([128, 8], bf16)
            with tc.tile_pool(name="pmlp", bufs=7, space="PSUM") as pmlp:
                for m in range(8):
                    hp = pmlp.tile([128, 1], f32, tag="hp")
                    for k in range(4):
                        nc.tensor.matmul(
                            out=hp[:],
                            lhsT=w1t_sb[:, k, m, :],
                            rhs=r_sb[:, k : k + 1],
                            start=(k == 0),
                            stop=(k == 3),
                        )
                    nc.scalar.activation(
                        out=h_sb[:, m : m + 1],
                        in_=hp[:],
                        func=mybir.ActivationFunctionType.Tanh,
                        bias=b1t_sb[:, m : m + 1],
                    )
                o_ps = pmlp.tile([2, 1], f32, tag="hp")
                for k in range(8):
                    nc.tensor.matmul(
                        out=o_ps[:],
                        lhsT=w2t_sb[:, k, :],
                        rhs=h_sb[:, k : k + 1],
                        start=(k == 0),
                        stop=(k == 7),
                    )
                o_sb = cpool.tile([2, 1], f32)
                nc.vector.tensor_tensor(
                    out=o_sb[:], in0=o_ps[:], in1=b2t_sb[:], op=mybir.AluOpType.add
                )
                nc.sync.dma_start(out=out[:], in_=o_sb[:])

    nc.finalize()
    return nc


def prep_in_maps(words, words_in_char, word_emb, chr_emb, conv_chr_w, conv_chr_b,
                 conv_sent_w, conv_sent_b, w1, b1, w2, b2):
    W = words.shape[0]
    WPC, NW, S, G = _shapes(W)

    words = np.asarray(words, np.int32)
    chars = np.asarray(words_in_char, np.int32)
    word_emb = np.asarray(word_emb, np.float32)
    chr_emb = np.asarray(chr_emb, np.float32)
    conv_chr_w = np.asarray(conv_chr_w, np.float32)
    conv_chr_b = np.asarray(conv_chr_b, np.float32)
    conv_sent_w = np.asarray(conv_sent_w, np.float32)
    conv_sent_b = np.asarray(conv_sent_b, np.float32)
    w1 = np.asarray(w1, np.float32)
    b1 = np.asarray(b1, np.float32)
    w2 = np.asarray(w2, np.float32)
    b2 = np.asarray(b2, np.float32)

    # host-folded char response tables (x64 into fp8 range)
    ET = [
        (ETS * (chr_emb @ conv_chr_w[:, :, k].T)).astype(FP8).astype(np.float32)
        for k in range(3)
    ]  # [c, dout]
    # DoubleRow weights: slot pairs with rhs k-tile (0=cE stream, 1=cO stream)
    #   par=0 (even outputs): slots (ET1, ET2); normal tap = ET0 on Le
    #   par=1 (odd outputs):  slots (ET0, ET1); normal tap = ET2 on Re
    slot_k = [(1, 2), (0, 1)]
    if SWI:
        wdr = np.zeros((128, 2, 2, 256), np.float32)
        for par in range(2):
            for m in range(2):
                a = ET[slot_k[par][0]][:, m * 128 : (m + 1) * 128]
                b = ET[slot_k[par][1]][:, m * 128 : (m + 1) * 128]
                wdr[:, par, m, :] = np.stack(
                    [a[:, ::-1], b[:, ::-1]], axis=2
                ).reshape(128, 256)
    else:
        wdr = np.zeros((128, 2, 2, 2, 128), np.float32)
        for par in range(2):
            for sl in range(2):
                k = slot_k[par][sl]
                wdr[:, par, sl, 0, :] = ET[k][:, :128]
                wdr[:, par, sl, 1, :] = ET[k][:, 128:]
    wn = np.zeros((128, 2, 2, 128), np.float32)
    for m in range(2):
        wn[:, 0, m, :] = ET[0][:, m * 128 : (m + 1) * 128]
        wn[:, 1, m, :] = ET[2][:, m * 128 : (m + 1) * 128]

    # char bias carries the x64 scale of u's char half
    cbias = np.ascontiguousarray(ETS * conv_chr_b.reshape(2, 128).T).astype(np.float32)
    # sentence conv: char-half input channels absorb the 1/64
    ws = conv_sent_w.copy()
    ws[:, D:, :] /= ETS
    wsT = np.ascontiguousarray(
        ws.transpose(1, 2, 0).reshape(4, 128, 3, 2 * D).transpose(1, 2, 0, 3)
    ).astype(BF16)                                        # [p, k, kc, c2]
    bsent = np.ascontiguousarray(conv_sent_b.reshape(4, 128).T).astype(np.float32)
    w1t = np.ascontiguousarray(
        w1.reshape(8, 128, 4, 128).transpose(3, 2, 0, 1)
    ).astype(BF16)                                  # [p, k, m, c]
    b1t = np.ascontiguousarray(b1.reshape(8, 128).T).astype(np.float32)
    w2t = np.ascontiguousarray(
        w2.T.reshape(8, 128, 2).transpose(1, 0, 2)
    ).astype(BF16)                                  # [p, k, j]
    b2t = b2.reshape(2, 1).astype(np.float32)
    ident = np.eye(128, dtype=np.float32)

    scol = np.arange(S)
    in_maps = []
    for c in range(CORES):
        lo = c * WPC - 1
        idxs = np.arange(lo, lo + NW)
        valid = (idxs >= 0) & (idxs < W)
        w_ext = np.where(valid, words[np.clip(idxs, 0, W - 1)], 0).astype(np.int32)
        ch_ext = np.zeros((NW, L), np.int32)
        ch_ext[valid] = chars[np.clip(idxs, 0, W - 1)[valid]]

        cE = ch_ext[:, 0::2]                               # [NW, 16]
        cO = ch_ext[:, 1::2]
        oh = np.zeros((128, 2, S), FP8)
        oh[cE.reshape(-1), 0, scol] = 1.0
        oh[cO.reshape(-1), 1, scol] = 1.0

        wpad = np.zeros(G * 128, np.int32)
        wpad[:NW] = w_ext
        widx = np.ascontiguousarray(wpad.reshape(G, 128).T)

        hsc = np.ones((128, 2), np.float32)
        if c == 0:
            hsc[:, 0] = 0.0
        if c == CORES - 1:
            hsc[:, 1] = 0.0

        in_maps.append(
            dict(
                onehot=oh,
                wdr=wdr.astype(FP8),
                wn=wn.astype(FP8),
                widx=widx,
                wemb=word_emb,
                cbias=cbias,
                wsT=wsT,
                bsent=bsent,
                w1t=w1t,
                b1t=b1t,
                w2t=w2t,
                b2t=b2t,
                hsc=hsc.astype(BF16),
                ident=ident,
            )
        )
    return in_maps


_CACHE = {}


def _get_nc(W):
    if W not in _CACHE:
        _CACHE[W] = build(W)
    return _CACHE[W]


def run(inputs, trace=False):
    W = np.asarray(inputs["words"]).shape[0]
    nc = _get_nc(W)
    in_maps = prep_in_maps(**inputs)
    res = run_bass_kernel_spmd(nc, in_maps, list(range(CORES)), trace=trace)
    out = np.asarray(res.results[0]["out"], np.float32).reshape(1, 2)
    return out, res


def kernel(**inputs) -> np.ndarray:
    out, _ = run(inputs, trace=False)
    return out
